# revision 12
# baseline (speedup 1.0000x reference)
"""DiffAttnV2-like fused kernel for Trainium2 (8 NeuronCores), v2.

Sharding: core = 4*b + g  (b = batch 0..1, g = head-group 0..3, 4 heads each).
Each core computes its 4 output heads' attention and a partial out = y_g @ Wo_g;
host sums the 4 partials per batch.

v2 changes vs v1 (820 us):
  - bf16 weights/x/q/k/v/y storage (precision sim: ~4e-3 max rel, gate 2e-2)
  - causal mask via gpsimd.affine_select on exp output (drops mask matmuls)
  - combine: lam*recip folded on rows + two rank-1 PE broadcasts + 3 DVE passes
  - pair-packed projection PSUMs (bufs=2 rotation hides evacuation latency)
  - deeper weight prefetch, double-buffered xTh/qTh
"""
import sys
sys.path.insert(0, "/opt/trn_rl_repo")
from contextlib import ExitStack

import numpy as np
import ml_dtypes

from concourse import bacc, mybir, tile
from concourse.bass_utils import run_bass_kernel_spmd

B, T, D, H = 2, 2048, 2048, 16
HPC = 4               # heads per core
NC = 8                # cores
NDC = D // 128        # 16 contraction chunks
NPH = 4               # t-phases
PT = T // NPH         # 512 t-cols per phase
SCALE = 1.0 / float(np.sqrt(D // H))

f32 = mybir.dt.float32
f32r = mybir.dt.float32r
bf16 = mybir.dt.bfloat16
EXP = mybir.ActivationFunctionType.Exp
SIG = mybir.ActivationFunctionType.Sigmoid
GE = mybir.AluOpType.is_ge
BF = ml_dtypes.bfloat16

_CACHE = {}

PSUM2 = False  # DVE cannot read two PSUM banks in one op (NCC_IBVF027)


def _build():
    nc = bacc.Bacc("TRN2", target_bir_lowering=False, debug=False)
    xTp = nc.dram_tensor("xTp", [NPH, 128, NDC, PT], bf16, kind="ExternalInput").ap()
    wqp = nc.dram_tensor("wqp", [4, 128, NDC, 256], bf16, kind="ExternalInput").ap()
    wkp = nc.dram_tensor("wkp", [2, 128, NDC, 256], bf16, kind="ExternalInput").ap()
    wvp = nc.dram_tensor("wvp", [2, 128, NDC, 256], bf16, kind="ExternalInput").ap()
    wlamp = nc.dram_tensor("wlamp", [128, NDC, HPC], bf16, kind="ExternalInput").ap()
    wop = nc.dram_tensor("wop", [4, 128, HPC, 512], bf16, kind="ExternalInput").ap()
    onesin = nc.dram_tensor("onesin", [128, 4], bf16, kind="ExternalInput").ap()
    selin = nc.dram_tensor("selin", [HPC, 512], f32r, kind="ExternalInput").ap()
    sel2in = nc.dram_tensor("sel2in", [2, 256], f32r, kind="ExternalInput").ap()
    out = nc.dram_tensor("out", [T, D], f32, kind="ExternalOutput").ap()

    with tile.TileContext(nc) as tc, ExitStack() as ctx:
        ctx.enter_context(nc.allow_low_precision(reason="bf16/fp32r pipeline"))
        persist = ctx.enter_context(tc.tile_pool(name="persist", bufs=1))
        xpool = ctx.enter_context(tc.tile_pool(name="xpool", bufs=2))
        qpool = ctx.enter_context(tc.tile_pool(name="qpool", bufs=2))
        wpool = ctx.enter_context(tc.tile_pool(name="wpool", bufs=3))
        wvpool = ctx.enter_context(tc.tile_pool(name="wvpool", bufs=2))
        epool = ctx.enter_context(tc.tile_pool(name="epool", bufs=3))
        cpool = ctx.enter_context(tc.tile_pool(name="cpool", bufs=2))
        ypool = ctx.enter_context(tc.tile_pool(name="ypool", bufs=2))
        opool = ctx.enter_context(tc.tile_pool(name="opool", bufs=2))
        # PSUM banks: pps s2[128,2,512]x2 = 4, ppy [128,512]x2 = 2,
        #             ppd [2,512]x1 = 1, pptr [128,512]x1 = 1  -> 8
        pps = ctx.enter_context(tc.tile_pool(name="pps", bufs=2, space="PSUM"))
        ppy = ctx.enter_context(tc.tile_pool(name="ppy", bufs=2, space="PSUM"))
        ppd = ctx.enter_context(tc.tile_pool(name="ppd", bufs=1, space="PSUM"))
        pptr = ctx.enter_context(tc.tile_pool(name="pptr", bufs=1, space="PSUM"))

        kT = persist.tile([128, HPC, T], bf16)            # 16KB
        vn = persist.tile([128, 2, NDC, 2, 128], bf16)    # 16KB [tk,(pair,tkc,pj),d]
        ones2 = persist.tile([128, 2, 2], bf16)           # den-row selectors
        nc.sync.dma_start(out=ones2.rearrange("p a b -> p (a b)"), in_=onesin[:])
        sel = persist.tile([HPC, HPC, 128], f32r)         # head-row selectors
        nc.sync.dma_start(out=sel.rearrange("p a b -> p (a b)"), in_=selin[:])
        sel2 = persist.tile([2, 2, 128], f32r)            # den-row bcast selectors
        nc.sync.dma_start(out=sel2.rearrange("p a b -> p (a b)"), in_=sel2in[:])

        def emit_wo(phw, yhw):
            t0w = PT * phw
            for dout in range(4):
                wo4 = wvpool.tile([128, HPC, 512], bf16, name=f"wo{phw}_{dout}",
                                  tag="wo4")
                nc.sync.dma_start(out=wo4[:], in_=wop[dout])
                for tsub in range(4):
                    alt = (dout * 4 + tsub) % 3
                    if alt < 2:
                        ps_o = ppy.tile([128, 512], f32,
                                        name=f"pso{phw}_{dout}_{tsub}", tag="y")
                    else:
                        ps_o = pptr.tile([128, 512], f32,
                                         name=f"pso{phw}_{dout}_{tsub}", tag="tr")
                    for hl in range(HPC):
                        nc.tensor.matmul(
                            ps_o[:], yhw[:, hl, 128 * tsub:128 * (tsub + 1)],
                            wo4[:, hl], start=(hl == 0), stop=(hl == HPC - 1))
                    ob = opool.tile([128, 512], f32,
                                    name=f"ob{phw}_{dout}_{tsub}", tag="ob")
                    if (dout * 4 + tsub) % 2 == 0:
                        nc.vector.tensor_copy(ob[:], ps_o[:])
                    else:
                        nc.scalar.copy(ob[:], ps_o[:])
                    nc.sync.dma_start(
                        out=out[t0w + 128 * tsub:t0w + 128 * (tsub + 1),
                                512 * dout:512 * (dout + 1)],
                        in_=ob[:])

        prev_wo = None
        for ph in range(NPH):
            t0 = PT * ph
            # ---- x^T slice for this phase ----
            xTh = xpool.tile([128, NDC, PT], bf16, name=f"xTh{ph}", tag="xTh")
            nc.sync.dma_start(out=xTh[:], in_=xTp[ph])

            # ---- q projections (4 head-pairs) ----
            qTh = qpool.tile([128, 8, PT], bf16, name=f"qTh{ph}", tag="qTh")
            ctx_q = nc.named_scope(f"proj{ph}"); ctx_q.__enter__()
            for pq in range(4):
                wt = wpool.tile([128, NDC, 256], bf16, name=f"wq{ph}_{pq}",
                                tag="wq")
                nc.sync.dma_start(out=wt[:], in_=wqp[pq])
                ps = pps.tile([128, 2, PT], f32, name=f"psq{ph}_{pq}", tag="s2")
                for j in range(2):
                    for dc in range(NDC):
                        nc.tensor.matmul(ps[:, j], wt[:, dc, 128 * j:128 * (j + 1)],
                                         xTh[:, dc],
                                         start=(dc == 0), stop=(dc == NDC - 1))
                nc.scalar.copy(qTh[:, 2 * pq:2 * pq + 2], ps[:])

            # ---- k projections (2 pairs) ----
            for pk in range(2):
                wt = wpool.tile([128, NDC, 256], bf16, name=f"wk{ph}_{pk}",
                                tag="wq")
                nc.sync.dma_start(out=wt[:], in_=wkp[pk])
                ps = pps.tile([128, 2, PT], f32, name=f"psk{ph}_{pk}", tag="s2")
                for j in range(2):
                    for dc in range(NDC):
                        nc.tensor.matmul(ps[:, j], wt[:, dc, 128 * j:128 * (j + 1)],
                                         xTh[:, dc],
                                         start=(dc == 0), stop=(dc == NDC - 1))
                nc.vector.tensor_copy(kT[:, 2 * pk:2 * pk + 2, t0:t0 + PT], ps[:])

            # ---- lam projection + sigmoid ----
            wlt = wpool.tile([128, NDC, HPC], bf16, name=f"wl{ph}", tag="wl",
                             bufs=1)
            nc.sync.dma_start(out=wlt[:], in_=wlamp[:])
            psl = pptr.tile([128, PT], f32, name=f"psl{ph}", tag="tr")
            for dc in range(NDC):
                nc.tensor.matmul(psl[0:HPC, :], wlt[:, dc], xTh[:, dc],
                                 start=(dc == 0), stop=(dc == NDC - 1))
            lamE = cpool.tile([HPC, PT], f32, name=f"lamE{ph}", tag="lamE",
                              bufs=1)
            nc.scalar.activation(lamE[:], psl[0:HPC, :], EXP, scale=-1.0)
            nc.vector.tensor_scalar_add(lamE[:], lamE[:], 1.0)
            lamF = cpool.tile([HPC, PT], f32, name=f"lamF{ph}", tag="lamF",
                              bufs=1)
            nc.vector.reciprocal_approx_fast(lamF[:], lamE[:])
            lamS = cpool.tile([HPC, PT], f32r, name=f"lam{ph}", tag="lam", bufs=1)
            nc.vector.tensor_copy(lamS[:], lamF[:])

            # ---- v projections (natural [tk, d]) ----
            for pair in range(2):
                wt = wvpool.tile([128, NDC, 256], bf16, name=f"wv{ph}_{pair}",
                                 tag="wv")
                nc.sync.dma_start(out=wt[:], in_=wvp[pair])
                for tg in range(2):  # tsub groups of 2
                    ps = pps.tile([128, 2, PT], f32, name=f"psv{ph}_{pair}_{tg}",
                                  tag="s2")
                    for t in range(2):
                        tsub = 2 * tg + t
                        for dc in range(NDC):
                            nc.tensor.matmul(
                                ps[:, t, 0:256],
                                xTh[:, dc, 128 * tsub:128 * (tsub + 1)],
                                wt[:, dc], start=(dc == 0), stop=(dc == NDC - 1))
                    nc.vector.tensor_copy(
                        vn[:, pair, 4 * ph + 2 * tg:4 * ph + 2 * tg + 2],
                        ps[:, :, 0:256])

            ctx_q.__exit__(None, None, None)
            # ---- Wo of previous phase (fills proj-evac stall window) ----
            if prev_wo is not None:
                with nc.named_scope(f"wo{ph-1}"):
                    emit_wo(*prev_wo)

            # ---- attention: 4 head-pairs ----
            ntk = 4 * (ph + 1)
            yh = ypool.tile([128, HPC, PT], bf16, name=f"yh{ph}", tag="yh")
            pending_combine = None
            ctx_a = nc.named_scope(f"attn{ph}"); ctx_a.__enter__()
            for hl in range(HPC):
                meta = []
                for j, qh in ((0, hl), (1, 4 + hl)):
                    khl = (hl // 2) if j == 0 else (2 + hl // 2)
                    meta.append((qh, khl, khl // 2, khl % 2))
                ps_y = [ppy.tile([128, PT], f32, name=f"psy{ph}_{hl}_{j}",
                                 tag="y") for j in range(2)]
                ps_den = ppd.tile([2, PT], f32, name=f"psd{ph}_{hl}", tag="den")

                def consume(bt, exs):
                    for j in range(2):
                        _, _, pair, pj = meta[j]
                        for cc in range(2):
                            tkc = 2 * bt + cc
                            exc = exs[j][:, cc]
                            nc.tensor.matmul(ps_den[0:2, :], ones2[:, j], exc,
                                             start=(j == 0 and tkc == 0),
                                             stop=(j == 1 and tkc == ntk - 1))
                            nc.tensor.matmul(ps_y[j][:], vn[:, pair, tkc, pj], exc,
                                             start=(tkc == 0), stop=(tkc == ntk - 1))

                pend = []
                for bt in range(ntk // 2):
                    if bt == 1 and pending_combine is not None:
                        pending_combine()
                        pending_combine = None
                    exs = []
                    for j in range(2):
                        qh, khl = meta[j][0], meta[j][1]
                        ps_s = pps.tile([128, 2, PT], f32,
                                        name=f"pss{ph}_{hl}_{bt}_{j}", tag="s2")
                        for cc in range(2):
                            tkc = 2 * bt + cc
                            nc.tensor.matmul(
                                ps_s[:, cc],
                                kT[:, khl, 128 * tkc:128 * (tkc + 1)],
                                qTh[:, qh], start=True, stop=True)
                        ex = epool.tile([128, 2, PT], bf16,
                                        name=f"ex{ph}_{hl}_{bt}_{j}", tag="ex",
                                        bufs=6)
                        nc.scalar.activation(ex[:], ps_s[:], EXP, scale=SCALE)
                        for cc in range(2):
                            tkc = 2 * bt + cc
                            o = 128 * tkc - t0
                            if o >= 0:  # diagonal chunk: zero the future
                                w = o + 128
                                nc.gpsimd.affine_select(
                                    ex[:, cc, 0:w], ex[:, cc, 0:w],
                                    base=-o, channel_multiplier=-1,
                                    pattern=[[1, w]], compare_op=GE, fill=0.0)
                        exs.append(ex)
                    pend.append((bt, exs))
                    if len(pend) > 2:
                        consume(*pend.pop(0))
                for p in pend:
                    consume(*p)

                # combine y_h = psy0*(1/d0) - lam*psy1*(1/d1); deferred
                rd = cpool.tile([2, PT], f32, name=f"rd{ph}_{hl}", tag="rd")
                nc.vector.reciprocal_approx_fast(rd[:], ps_den[0:2, :])
                rden2 = cpool.tile([2, PT], f32r, name=f"rden{ph}_{hl}",
                                   tag="rden")
                nc.scalar.copy(rden2[:], rd[:])

                def _combine(hl=hl, ps_y=ps_y, rden2=rden2):
                    ts = []
                    for j in range(2):
                        pb = pptr.tile([128, PT], f32, name=f"pb{ph}_{hl}_{j}",
                                       tag="tr")
                        nc.tensor.matmul(pb[:], sel2[:, j], rden2[:],
                                         start=True, stop=True)
                        tj = cpool.tile([128, PT], f32, name=f"t{j}_{ph}_{hl}",
                                        tag=f"t{j}")
                        if PSUM2:
                            nc.vector.tensor_mul(tj[:], ps_y[j][:], pb[:])
                        else:
                            pbs = cpool.tile([128, PT], f32,
                                             name=f"pbs{ph}_{hl}_{j}", tag="pbs")
                            nc.scalar.copy(pbs[:], pb[:])
                            nc.vector.tensor_mul(tj[:], ps_y[j][:], pbs[:])
                        ts.append(tj)
                    ps_lam = pptr.tile([128, PT], f32, name=f"pslam{ph}_{hl}",
                                       tag="tr")
                    nc.tensor.matmul(ps_lam[:], sel[:, hl], lamS[:],
                                     start=True, stop=True)
                    nc.vector.tensor_mul(ts[1][:], ts[1][:], ps_lam[:])
                    nc.vector.tensor_sub(yh[:, hl], ts[0][:], ts[1][:])

                if hl < HPC - 1 and ntk >= 4:
                    pending_combine = _combine
                else:
                    _combine()

            ctx_a.__exit__(None, None, None)
            prev_wo = (ph, yh)
        with nc.named_scope("wo3"):
            emit_wo(*prev_wo)
    nc.compile()
    return nc


def _get_nc():
    if "nc" not in _CACHE:
        _CACHE["nc"] = _build()
    return _CACHE["nc"]


def kernel(x, Wq1, Wq2, Wk, Wv, Wlam, Wo, **_ignored):
    x = np.ascontiguousarray(np.asarray(x, dtype=np.float32))
    Wq1 = np.asarray(Wq1, dtype=np.float32)
    Wq2 = np.asarray(Wq2, dtype=np.float32)
    Wk = np.asarray(Wk, dtype=np.float32)
    Wv = np.asarray(Wv, dtype=np.float32)
    Wlam = np.asarray(Wlam, dtype=np.float32)
    Wo = np.asarray(Wo, dtype=np.float32)

    ones2 = np.zeros((128, 2, 2), dtype=np.float32)
    ones2[:, 0, 0] = 1.0
    ones2[:, 1, 1] = 1.0
    ones2 = ones2.reshape(128, 4)
    sel2 = np.zeros((2, 2, 128), dtype=np.float32)
    sel2[0, 0, :] = 1.0
    sel2[1, 1, :] = 1.0
    sel2 = sel2.reshape(2, 256)
    selv = np.zeros((HPC, HPC, 128), dtype=np.float32)
    for i in range(HPC):
        selv[i, i, :] = 1.0
    selv = selv.reshape(HPC, 512)

    xTs = []
    for b in range(B):
        xt = x[b].T                                   # [D, T]
        xTs.append(np.ascontiguousarray(
            xt.reshape(NDC, 128, NPH, PT).transpose(2, 1, 0, 3).astype(BF)))

    in_maps = []
    for core in range(NC):
        b, g = divmod(core, 4)
        kv_cols = np.r_[256 * g:256 * g + 256, 1024 + 256 * g:1024 + 256 * g + 256]
        wq_s = np.concatenate([Wq1[:, 512 * g:512 * (g + 1)],
                               Wq2[:, 512 * g:512 * (g + 1)]], axis=1)  # [D, 1024]
        wqp_v = np.ascontiguousarray(
            wq_s.reshape(NDC, 128, 4, 256).transpose(2, 1, 0, 3).astype(BF))
        wk_s = Wk[:, kv_cols]
        wkp_v = np.ascontiguousarray(
            wk_s.reshape(NDC, 128, 2, 256).transpose(2, 1, 0, 3).astype(BF))
        wv_s = Wv[:, kv_cols]
        wvp_v = np.ascontiguousarray(
            wv_s.reshape(NDC, 128, 2, 256).transpose(2, 1, 0, 3).astype(BF))
        wlam_s = Wlam[:, 4 * g:4 * (g + 1)]
        wlamp_v = np.ascontiguousarray(
            wlam_s.reshape(NDC, 128, HPC).transpose(1, 0, 2).astype(BF))
        wo_s = Wo[512 * g:512 * (g + 1), :]
        wop_v = np.ascontiguousarray(
            wo_s.reshape(HPC, 128, 4, 512).transpose(2, 1, 0, 3).astype(BF))
        in_maps.append({
            "xTp": xTs[b],
            "wqp": wqp_v,
            "wkp": wkp_v,
            "wvp": wvp_v,
            "wlamp": wlamp_v,
            "wop": wop_v,
            "onesin": ones2.astype(BF),
            "selin": selv,
            "sel2in": sel2,
        })

    last_exc = None
    for attempt in range(3):
        try:
            res = run_bass_kernel_spmd(_get_nc(), in_maps, list(range(NC)),
                                       **_CACHE.get("run_kwargs", {}))
            break
        except Exception as e:  # transient NRT device wedges recover on retry
            last_exc = e
            _CACHE.pop("nc", None)
            import time as _time
            _time.sleep(5)
    else:
        raise last_exc
    _CACHE["last_res"] = res
    out = np.zeros((B, T, D), dtype=np.float32)
    for core in range(NC):
        out[core // 4] += res.results[core]["out"]
    return out


# revision 13
# speedup vs baseline: 1.0219x; 1.0219x over previous
"""DiffAttnV2-like fused kernel for Trainium2 (8 NeuronCores), v2.

Sharding: core = 4*b + g  (b = batch 0..1, g = head-group 0..3, 4 heads each).
Each core computes its 4 output heads' attention and a partial out = y_g @ Wo_g;
host sums the 4 partials per batch.

v2 changes vs v1 (820 us):
  - bf16 weights/x/q/k/v/y storage (precision sim: ~4e-3 max rel, gate 2e-2)
  - causal mask via gpsimd.affine_select on exp output (drops mask matmuls)
  - combine: lam*recip folded on rows + two rank-1 PE broadcasts + 3 DVE passes
  - pair-packed projection PSUMs (bufs=2 rotation hides evacuation latency)
  - deeper weight prefetch, double-buffered xTh/qTh
"""
import sys
sys.path.insert(0, "/opt/trn_rl_repo")
from contextlib import ExitStack

import numpy as np
import ml_dtypes

from concourse import bacc, mybir, tile
from concourse.bass_utils import run_bass_kernel_spmd

B, T, D, H = 2, 2048, 2048, 16
HPC = 4               # heads per core
NC = 8                # cores
NDC = D // 128        # 16 contraction chunks
NPH = 4               # t-phases
PT = T // NPH         # 512 t-cols per phase
SCALE = 1.0 / float(np.sqrt(D // H))

f32 = mybir.dt.float32
f32r = mybir.dt.float32r
bf16 = mybir.dt.bfloat16
EXP = mybir.ActivationFunctionType.Exp
SIG = mybir.ActivationFunctionType.Sigmoid
GE = mybir.AluOpType.is_ge
BF = ml_dtypes.bfloat16

_CACHE = {}

PSUM2 = False  # DVE cannot read two PSUM banks in one op (NCC_IBVF027)


def _build():
    nc = bacc.Bacc("TRN2", target_bir_lowering=False, debug=False)
    xTp = nc.dram_tensor("xTp", [NPH, 128, NDC, PT], bf16, kind="ExternalInput").ap()
    wqp = nc.dram_tensor("wqp", [4, 128, NDC, 256], bf16, kind="ExternalInput").ap()
    wkp = nc.dram_tensor("wkp", [2, 128, NDC, 256], bf16, kind="ExternalInput").ap()
    wvp = nc.dram_tensor("wvp", [2, 128, NDC, 256], bf16, kind="ExternalInput").ap()
    wlamp = nc.dram_tensor("wlamp", [128, NDC, HPC], bf16, kind="ExternalInput").ap()
    wop = nc.dram_tensor("wop", [4, 128, HPC, 512], bf16, kind="ExternalInput").ap()
    onesin = nc.dram_tensor("onesin", [128, 4], bf16, kind="ExternalInput").ap()
    selin = nc.dram_tensor("selin", [HPC, 512], f32r, kind="ExternalInput").ap()
    sel2in = nc.dram_tensor("sel2in", [2, 256], f32r, kind="ExternalInput").ap()
    out = nc.dram_tensor("out", [T, D], f32, kind="ExternalOutput").ap()

    with tile.TileContext(nc) as tc, ExitStack() as ctx:
        ctx.enter_context(nc.allow_low_precision(reason="bf16/fp32r pipeline"))
        persist = ctx.enter_context(tc.tile_pool(name="persist", bufs=1))
        xpool = ctx.enter_context(tc.tile_pool(name="xpool", bufs=2))
        qpool = ctx.enter_context(tc.tile_pool(name="qpool", bufs=2))
        wpool = ctx.enter_context(tc.tile_pool(name="wpool", bufs=3))
        wvpool = ctx.enter_context(tc.tile_pool(name="wvpool", bufs=2))
        epool = ctx.enter_context(tc.tile_pool(name="epool", bufs=3))
        cpool = ctx.enter_context(tc.tile_pool(name="cpool", bufs=2))
        ypool = ctx.enter_context(tc.tile_pool(name="ypool", bufs=2))
        opool = ctx.enter_context(tc.tile_pool(name="opool", bufs=2))
        # PSUM banks: pps s2[128,2,512]x2 = 4, ppy [128,512]x2 = 2,
        #             ppd [2,512]x1 = 1, pptr [128,512]x1 = 1  -> 8
        pps = ctx.enter_context(tc.tile_pool(name="pps", bufs=2, space="PSUM"))
        ppy = ctx.enter_context(tc.tile_pool(name="ppy", bufs=2, space="PSUM"))
        ppd = ctx.enter_context(tc.tile_pool(name="ppd", bufs=1, space="PSUM"))
        pptr = ctx.enter_context(tc.tile_pool(name="pptr", bufs=1, space="PSUM"))

        kT = persist.tile([128, HPC, T], bf16)            # 16KB
        vn = persist.tile([128, 2, NDC, 2, 128], bf16)    # 16KB [tk,(pair,tkc,pj),d]
        ones2 = persist.tile([128, 2, 2], bf16)           # den-row selectors
        nc.sync.dma_start(out=ones2.rearrange("p a b -> p (a b)"), in_=onesin[:])
        sel = persist.tile([HPC, HPC, 128], f32r)         # head-row selectors
        nc.sync.dma_start(out=sel.rearrange("p a b -> p (a b)"), in_=selin[:])
        sel2 = persist.tile([2, 2, 128], f32r)            # den-row bcast selectors
        nc.sync.dma_start(out=sel2.rearrange("p a b -> p (a b)"), in_=sel2in[:])

        def emit_wo(phw, yhw):
            t0w = PT * phw
            for dout in range(4):
                wo4 = wvpool.tile([128, HPC, 512], bf16, name=f"wo{phw}_{dout}",
                                  tag="wo4")
                nc.sync.dma_start(out=wo4[:], in_=wop[dout])
                for tsub in range(4):
                    alt = (dout * 4 + tsub) % 3
                    if alt < 2:
                        ps_o = ppy.tile([128, 512], f32,
                                        name=f"pso{phw}_{dout}_{tsub}", tag="y")
                    else:
                        ps_o = pptr.tile([128, 512], f32,
                                         name=f"pso{phw}_{dout}_{tsub}", tag="tr")
                    for hl in range(HPC):
                        nc.tensor.matmul(
                            ps_o[:], yhw[:, hl, 128 * tsub:128 * (tsub + 1)],
                            wo4[:, hl], start=(hl == 0), stop=(hl == HPC - 1))
                    ob = opool.tile([128, 512], f32,
                                    name=f"ob{phw}_{dout}_{tsub}", tag="ob")
                    if (dout * 4 + tsub) % 2 == 0:
                        nc.vector.tensor_copy(ob[:], ps_o[:])
                    else:
                        nc.scalar.copy(ob[:], ps_o[:])
                    nc.sync.dma_start(
                        out=out[t0w + 128 * tsub:t0w + 128 * (tsub + 1),
                                512 * dout:512 * (dout + 1)],
                        in_=ob[:])

        prev_wo = None
        for ph in range(NPH):
            t0 = PT * ph
            # ---- x^T slice for this phase ----
            xTh = xpool.tile([128, NDC, PT], bf16, name=f"xTh{ph}", tag="xTh")
            nc.sync.dma_start(out=xTh[:], in_=xTp[ph])

            # ---- q projections (4 head-pairs) ----
            qTh = qpool.tile([128, 8, PT], bf16, name=f"qTh{ph}", tag="qTh")
            ctx_q = nc.named_scope(f"proj{ph}"); ctx_q.__enter__()
            for pq in range(4):
                wt = wpool.tile([128, NDC, 256], bf16, name=f"wq{ph}_{pq}",
                                tag="wq")
                nc.sync.dma_start(out=wt[:], in_=wqp[pq])
                ps = pps.tile([128, 2, PT], f32, name=f"psq{ph}_{pq}", tag="s2")
                for j in range(2):
                    for dc in range(NDC):
                        nc.tensor.matmul(ps[:, j], wt[:, dc, 128 * j:128 * (j + 1)],
                                         xTh[:, dc],
                                         start=(dc == 0), stop=(dc == NDC - 1))
                nc.scalar.copy(qTh[:, 2 * pq:2 * pq + 2], ps[:])

            # ---- k projections (2 pairs) ----
            for pk in range(2):
                wt = wpool.tile([128, NDC, 256], bf16, name=f"wk{ph}_{pk}",
                                tag="wq")
                nc.sync.dma_start(out=wt[:], in_=wkp[pk])
                ps = pps.tile([128, 2, PT], f32, name=f"psk{ph}_{pk}", tag="s2")
                for j in range(2):
                    for dc in range(NDC):
                        nc.tensor.matmul(ps[:, j], wt[:, dc, 128 * j:128 * (j + 1)],
                                         xTh[:, dc],
                                         start=(dc == 0), stop=(dc == NDC - 1))
                nc.vector.tensor_copy(kT[:, 2 * pk:2 * pk + 2, t0:t0 + PT], ps[:])

            # ---- lam projection + sigmoid ----
            wlt = wpool.tile([128, NDC, HPC], bf16, name=f"wl{ph}", tag="wl",
                             bufs=1)
            nc.sync.dma_start(out=wlt[:], in_=wlamp[:])
            psl = pptr.tile([128, PT], f32, name=f"psl{ph}", tag="tr")
            for dc in range(NDC):
                nc.tensor.matmul(psl[0:HPC, :], wlt[:, dc], xTh[:, dc],
                                 start=(dc == 0), stop=(dc == NDC - 1))
            lamE = cpool.tile([HPC, PT], f32, name=f"lamE{ph}", tag="lamE",
                              bufs=1)
            nc.scalar.activation(lamE[:], psl[0:HPC, :], EXP, scale=-1.0)
            nc.vector.tensor_scalar_add(lamE[:], lamE[:], 1.0)
            lamF = cpool.tile([HPC, PT], f32, name=f"lamF{ph}", tag="lamF",
                              bufs=1)
            nc.vector.reciprocal_approx_fast(lamF[:], lamE[:])
            lamS = cpool.tile([HPC, PT], f32r, name=f"lam{ph}", tag="lam", bufs=1)
            nc.vector.tensor_copy(lamS[:], lamF[:])

            # ---- v projections (natural [tk, d]) ----
            for pair in range(2):
                wt = wvpool.tile([128, NDC, 256], bf16, name=f"wv{ph}_{pair}",
                                 tag="wv")
                nc.sync.dma_start(out=wt[:], in_=wvp[pair])
                for tg in range(2):  # tsub groups of 2
                    ps = pps.tile([128, 2, PT], f32, name=f"psv{ph}_{pair}_{tg}",
                                  tag="s2")
                    for t in range(2):
                        tsub = 2 * tg + t
                        for dc in range(NDC):
                            nc.tensor.matmul(
                                ps[:, t, 0:256],
                                xTh[:, dc, 128 * tsub:128 * (tsub + 1)],
                                wt[:, dc], start=(dc == 0), stop=(dc == NDC - 1))
                    nc.vector.tensor_copy(
                        vn[:, pair, 4 * ph + 2 * tg:4 * ph + 2 * tg + 2],
                        ps[:, :, 0:256])

            ctx_q.__exit__(None, None, None)
            # ---- Wo of previous phase (fills proj-evac stall window) ----
            if prev_wo is not None:
                with nc.named_scope(f"wo{ph-1}"):
                    emit_wo(*prev_wo)

            # ---- attention: 4 head-pairs ----
            ntk = 4 * (ph + 1)
            yh = ypool.tile([128, HPC, PT], bf16, name=f"yh{ph}", tag="yh")
            pending_combine = None
            ctx_a = nc.named_scope(f"attn{ph}"); ctx_a.__enter__()
            for hl in range(HPC):
                meta = []
                for j, qh in ((0, hl), (1, 4 + hl)):
                    khl = (hl // 2) if j == 0 else (2 + hl // 2)
                    meta.append((qh, khl, khl // 2, khl % 2))
                ps_y = [ppy.tile([128, PT], f32, name=f"psy{ph}_{hl}_{j}",
                                 tag="y") for j in range(2)]
                ps_den = ppd.tile([2, PT], f32, name=f"psd{ph}_{hl}", tag="den")

                def consume(bt, exs):
                    for j in range(2):
                        _, _, pair, pj = meta[j]
                        for cc in range(2):
                            tkc = 2 * bt + cc
                            nc.tensor.matmul(ps_y[j][:], vn[:, pair, tkc, pj],
                                             exs[j][:, cc],
                                             start=(tkc == 0), stop=(tkc == ntk - 1))
                    for j in range(2):
                        for cc in range(2):
                            tkc = 2 * bt + cc
                            nc.tensor.matmul(ps_den[0:2, :], ones2[:, j],
                                             exs[j][:, cc],
                                             start=(j == 0 and tkc == 0),
                                             stop=(j == 1 and tkc == ntk - 1))

                pend = []
                for bt in range(ntk // 2):
                    if bt == 1 and pending_combine is not None:
                        pending_combine()
                        pending_combine = None
                    exs = []
                    for j in range(2):
                        qh, khl = meta[j][0], meta[j][1]
                        ps_s = pps.tile([128, 2, PT], f32,
                                        name=f"pss{ph}_{hl}_{bt}_{j}", tag="s2")
                        for cc in range(2):
                            tkc = 2 * bt + cc
                            nc.tensor.matmul(
                                ps_s[:, cc],
                                kT[:, khl, 128 * tkc:128 * (tkc + 1)],
                                qTh[:, qh], start=True, stop=True)
                        ex = epool.tile([128, 2, PT], bf16,
                                        name=f"ex{ph}_{hl}_{bt}_{j}", tag="ex",
                                        bufs=8)
                        nc.scalar.activation(ex[:], ps_s[:], EXP, scale=SCALE)
                        for cc in range(2):
                            tkc = 2 * bt + cc
                            o = 128 * tkc - t0
                            if o >= 0:  # diagonal chunk: zero the future
                                w = o + 128
                                nc.gpsimd.affine_select(
                                    ex[:, cc, 0:w], ex[:, cc, 0:w],
                                    base=-o, channel_multiplier=-1,
                                    pattern=[[1, w]], compare_op=GE, fill=0.0)
                        exs.append(ex)
                    pend.append((bt, exs))
                    if len(pend) > 3:
                        consume(*pend.pop(0))
                for p in pend:
                    consume(*p)

                # combine y_h = psy0*(1/d0) - lam*psy1*(1/d1); deferred
                rd = cpool.tile([2, PT], f32, name=f"rd{ph}_{hl}", tag="rd")
                nc.vector.reciprocal_approx_fast(rd[:], ps_den[0:2, :])
                rden2 = cpool.tile([2, PT], f32r, name=f"rden{ph}_{hl}",
                                   tag="rden")
                nc.scalar.copy(rden2[:], rd[:])

                def _combine(hl=hl, ps_y=ps_y, rden2=rden2):
                    ts = []
                    for j in range(2):
                        pb = pptr.tile([128, PT], f32, name=f"pb{ph}_{hl}_{j}",
                                       tag="tr")
                        nc.tensor.matmul(pb[:], sel2[:, j], rden2[:],
                                         start=True, stop=True)
                        tj = cpool.tile([128, PT], f32, name=f"t{j}_{ph}_{hl}",
                                        tag=f"t{j}")
                        if PSUM2:
                            nc.vector.tensor_mul(tj[:], ps_y[j][:], pb[:])
                        else:
                            pbs = cpool.tile([128, PT], f32,
                                             name=f"pbs{ph}_{hl}_{j}", tag="pbs")
                            nc.scalar.copy(pbs[:], pb[:])
                            nc.vector.tensor_mul(tj[:], ps_y[j][:], pbs[:])
                        ts.append(tj)
                    ps_lam = pptr.tile([128, PT], f32, name=f"pslam{ph}_{hl}",
                                       tag="tr")
                    nc.tensor.matmul(ps_lam[:], sel[:, hl], lamS[:],
                                     start=True, stop=True)
                    nc.vector.tensor_mul(ts[1][:], ts[1][:], ps_lam[:])
                    nc.vector.tensor_sub(yh[:, hl], ts[0][:], ts[1][:])

                if hl < HPC - 1 and ntk >= 4:
                    pending_combine = _combine
                else:
                    _combine()

            ctx_a.__exit__(None, None, None)
            prev_wo = (ph, yh)
        with nc.named_scope("wo3"):
            emit_wo(*prev_wo)
    nc.compile()
    return nc


def _get_nc():
    if "nc" not in _CACHE:
        _CACHE["nc"] = _build()
    return _CACHE["nc"]


def kernel(x, Wq1, Wq2, Wk, Wv, Wlam, Wo, **_ignored):
    x = np.ascontiguousarray(np.asarray(x, dtype=np.float32))
    Wq1 = np.asarray(Wq1, dtype=np.float32)
    Wq2 = np.asarray(Wq2, dtype=np.float32)
    Wk = np.asarray(Wk, dtype=np.float32)
    Wv = np.asarray(Wv, dtype=np.float32)
    Wlam = np.asarray(Wlam, dtype=np.float32)
    Wo = np.asarray(Wo, dtype=np.float32)

    ones2 = np.zeros((128, 2, 2), dtype=np.float32)
    ones2[:, 0, 0] = 1.0
    ones2[:, 1, 1] = 1.0
    ones2 = ones2.reshape(128, 4)
    sel2 = np.zeros((2, 2, 128), dtype=np.float32)
    sel2[0, 0, :] = 1.0
    sel2[1, 1, :] = 1.0
    sel2 = sel2.reshape(2, 256)
    selv = np.zeros((HPC, HPC, 128), dtype=np.float32)
    for i in range(HPC):
        selv[i, i, :] = 1.0
    selv = selv.reshape(HPC, 512)

    xTs = []
    for b in range(B):
        xt = x[b].T                                   # [D, T]
        xTs.append(np.ascontiguousarray(
            xt.reshape(NDC, 128, NPH, PT).transpose(2, 1, 0, 3).astype(BF)))

    in_maps = []
    for core in range(NC):
        b, g = divmod(core, 4)
        kv_cols = np.r_[256 * g:256 * g + 256, 1024 + 256 * g:1024 + 256 * g + 256]
        wq_s = np.concatenate([Wq1[:, 512 * g:512 * (g + 1)],
                               Wq2[:, 512 * g:512 * (g + 1)]], axis=1)  # [D, 1024]
        wqp_v = np.ascontiguousarray(
            wq_s.reshape(NDC, 128, 4, 256).transpose(2, 1, 0, 3).astype(BF))
        wk_s = Wk[:, kv_cols]
        wkp_v = np.ascontiguousarray(
            wk_s.reshape(NDC, 128, 2, 256).transpose(2, 1, 0, 3).astype(BF))
        wv_s = Wv[:, kv_cols]
        wvp_v = np.ascontiguousarray(
            wv_s.reshape(NDC, 128, 2, 256).transpose(2, 1, 0, 3).astype(BF))
        wlam_s = Wlam[:, 4 * g:4 * (g + 1)]
        wlamp_v = np.ascontiguousarray(
            wlam_s.reshape(NDC, 128, HPC).transpose(1, 0, 2).astype(BF))
        wo_s = Wo[512 * g:512 * (g + 1), :]
        wop_v = np.ascontiguousarray(
            wo_s.reshape(HPC, 128, 4, 512).transpose(2, 1, 0, 3).astype(BF))
        in_maps.append({
            "xTp": xTs[b],
            "wqp": wqp_v,
            "wkp": wkp_v,
            "wvp": wvp_v,
            "wlamp": wlamp_v,
            "wop": wop_v,
            "onesin": ones2.astype(BF),
            "selin": selv,
            "sel2in": sel2,
        })

    last_exc = None
    for attempt in range(3):
        try:
            res = run_bass_kernel_spmd(_get_nc(), in_maps, list(range(NC)),
                                       **_CACHE.get("run_kwargs", {}))
            break
        except Exception as e:  # transient NRT device wedges recover on retry
            last_exc = e
            _CACHE.pop("nc", None)
            import time as _time
            _time.sleep(5)
    else:
        raise last_exc
    _CACHE["last_res"] = res
    out = np.zeros((B, T, D), dtype=np.float32)
    for core in range(NC):
        out[core // 4] += res.results[core]["out"]
    return out
